# revision 2
# baseline (speedup 1.0000x reference)
"""Triangular matmul C = triu(triu(A) @ triu(B)) on 8 TRN2 NeuronCores.

Structure: the (I, K, J) block-tetrahedron {I <= K <= J} (128x128 blocks,
N=4096 -> 32 blocks/side) is sharded by output row-block I across the 8
cores with a work-balanced assignment.  Each core runs its own statically
addressed program inside a `tc.If(partition_id == c)` block.

Per core: row-blocks are processed in groups of <=4 that share one sweep
over the B strips (B[K, K*128:] for K >= min(group)).  The J axis is cut
into 512-wide phases; each group member I gets one PSUM bank per phase
(double-buffered), accumulating A^T[K,I] @ B[K, phase-window] over K, then
evicting to the output.

Numerics: fp32 operands are split on the host into bf16 (hi, lo) pairs and
each block product uses 3 bf16 matmuls (Ah@Bh + Ah@Bl + Al@Bh), giving
~5e-6 relative error vs the fp32 reference at 3/4 the cost of the PE's
native 4-pass fp32 mode (measured on HW: rel_absmax 4.8e-6 at N=4096).

DMA discipline: one ~256KB DMA per (K-strip, phase) carrying hi and lo
planes together (Bcat = [Bh | Bl]), issued alternately from the SP and DVE
sequencers; A^T strips are host-packed per core (apack) so each strip is a
few large-line DMAs on the GpSimd sequencer; PSUM evictions copy on ACT and
store from its sequencer.  This keeps ~16 DMA engines busy without
sequencer issue serialization.

The kernel takes FULL (unsharded) inputs and returns the FULL output.
"""

import numpy as np

N = 4096
BLK = 128
NB = N // BLK  # 32
N_CORES = 8
PHASE = 512  # J-phase width (one PSUM bank of fp32)
MODE = "bf16x3"  # "bf16x3" | "fp32r" | "fp32"

# Work-balanced assignment of row-blocks I to cores (work(I) = T(32-I),
# T(m)=m(m+1)/2; bins balanced to 743..752 of 5984/8=748).
BINS = [
    [0, 14, 23],
    [1, 15, 21, 25, 29],
    [2, 13, 20, 28],
    [4, 12, 16],
    [3, 10, 22],
    [6, 9, 17, 30],
    [5, 11, 19, 24, 27, 31],
    [7, 8, 18, 26],
]
MAXB = max(len(b) for b in BINS)  # output row-slots per core
# A-pack slot layout: per core, the A^T strips (one 128x128 block per slot,
# hi+lo planes) for each owned I, K = I..31, concatenated.
ABASE = [
    {I: int(np.cumsum([0] + [NB - J for J in sorted(b)])[i]) for i, I in enumerate(sorted(b))}
    for b in BINS
]
NSLOT = 80  # >= max per-core total blocks (75)
ACHUNK = 4  # A-load DMA granularity in k-blocks


def _groups(bin_is):
    """Split a sorted bin into contiguous groups of <=4 minimizing the
    total B-strip traffic sum(T(32 - min(group)))."""
    Is = sorted(bin_is)
    t = lambda m: m * (m + 1) // 2
    best = None

    def rec(i, acc, parts):
        nonlocal best
        if i == len(Is):
            if best is None or acc < best[0]:
                best = (acc, [list(p) for p in parts])
            return
        for g in range(1, 5):
            if i + g <= len(Is):
                rec(i + g, acc + t(NB - Is[i]), parts + [Is[i : i + g]])

    rec(0, 0, [])
    return best[1]


def _emit_core(nc, tc, pools, dram_io, core, mode, variant="full"):
    """K-major schedule: one row-block I at a time, full output row in PSUM
    (8 banks), K-sweep with each A-tile's weights amortized over all J-chunks
    (weight switches are ~180ns on HW; this gives 2 per (I,K) instead of 2
    per (I,K,phase))."""
    apool, bpool, cpool, psum_pool = pools
    import concourse.mybir as mybir

    f32 = mybir.dt.float32
    nplane = 2 if mode == "bf16x3" else 1
    dt_in = {
        "bf16x3": mybir.dt.bfloat16,
        "fp32r": mybir.dt.float32r,
        "fp32": f32,
    }[mode]
    apack, bcat, cpart = dram_io["apack"], dram_io["bcat"], dram_io["cpart"]
    bcat3 = bcat.rearrange("k (t n) -> k t n", t=nplane)

    bin_is = BINS[core]
    slot = {I: s for s, I in enumerate(sorted(bin_is))}
    bdma_engines = [nc.sync, nc.scalar]
    bdma_i = 0

    static_b = None
    if "nobdma" in variant:
        static_b = []
        for ci in range(NB // 8):
            sb_t = bpool.tile(
                [BLK, nplane, 2 * PHASE], dt_in, name=f"sb_{ci}", tag=f"sb{ci}", bufs=1
            )
            nc.gpsimd.memset(sb_t[:], 0.5)
            static_b.append(sb_t)

    for I in sorted(bin_is):
        nblk = NB - I
        base = ABASE[core][I]
        a_t = apool.tile([BLK, nblk, nplane, BLK], dt_in, name=f"a_{I}", tag="a")
        for j0 in range(0, nblk, ACHUNK):
            j1 = min(j0 + ACHUNK, nblk)
            nc.gpsimd.dma_start(
                a_t[:, j0:j1, :, :], apack[:, base + j0 : base + j1, :, :]
            )
        c0 = I // 4  # first active PSUM bank / J-chunk
        ps = {
            c: psum_pool.tile([BLK, PHASE], f32, name=f"ps_{I}_{c}", tag=f"ps{c}")
            for c in range(c0, NB // 4)
        }

        for K in range(I, NB):
            kb = K - I
            # B strip double-chunks (1024 cols -> 2KB DMA lines; hi+lo planes
            # in one DMA).  Each plane feeds two 512-wide matmuls (PSUM bank
            # limit).
            b_ts = {}
            for d in range(K // 8, NB // 8):
                pstart = max(K * BLK, 2 * PHASE * d)
                width = 2 * PHASE * (d + 1) - pstart
                if "nobdma" in variant:
                    b_ts[d] = (static_b[d], pstart, width)
                    continue
                b_t = bpool.tile(
                    [BLK, nplane, 2 * PHASE], dt_in, name=f"b_{K}_{d}", tag="b"
                )
                for t in range(nplane):
                    eng = bdma_engines[bdma_i % len(bdma_engines)]
                    bdma_i += 1
                    eng.dma_start(
                        b_t[:, t, :width],
                        bcat3[K * BLK : (K + 1) * BLK, t, pstart : pstart + width],
                    )
                b_ts[d] = (b_t, pstart, width)

            first = K == I
            if mode == "bf16x3":
                passes = [(0, 0), (0, 1), (1, 0)]  # (A plane, B plane)
            else:
                passes = [(0, 0)]
            for pi, (ta, tb) in enumerate(passes):
                a_w = a_t[:, kb, ta, :]
                for c in range(K // 4, NB // 4):
                    d = c // 2
                    b_t, pstart, width = b_ts[d]
                    cstart = max(pstart, PHASE * c)  # global col of this MM
                    cwidth = PHASE * (c + 1) - cstart
                    o = ps[c][:, cstart - PHASE * c : PHASE]
                    boff = cstart - pstart  # offset into the b tile
                    is_first = first and pi == 0
                    is_last = pi == len(passes) - 1 and K == min(4 * c + 3, NB - 1)
                    if "nomm" not in variant:
                        nc.tensor.matmul(
                            o, a_w, b_t[:, tb, boff : boff + cwidth],
                            start=is_first, stop=is_last,
                        )

        # Evict the full output row; next I's banks free up as copies drain.
        for c in range(c0, NB // 4):
            if "nomm" in variant and "noevict" in variant:
                continue
            coff0 = max(I * BLK - PHASE * c, 0)
            w = PHASE - coff0
            ct = cpool.tile([BLK, PHASE], f32, name=f"c_{I}_{c}", tag="cst")
            nc.vector.tensor_copy(ct[:, :w], ps[c][:, coff0:PHASE])
            r0 = slot[I] * BLK
            nc.gpsimd.dma_start(
                cpart[r0 : r0 + BLK, PHASE * c + coff0 : PHASE * (c + 1)],
                ct[:, :w],
            )


def _build(mode, repeat=1, variant="full"):
    import concourse.mybir as mybir
    import concourse.tile as tile
    from concourse import bacc

    nc = bacc.Bacc(None, target_bir_lowering=False, debug=False)
    f32 = mybir.dt.float32
    nplane = 2 if mode == "bf16x3" else 1
    dt_in = {
        "bf16x3": mybir.dt.bfloat16,
        "fp32r": mybir.dt.float32r,
        "fp32": f32,
    }[mode]
    with tile.TileContext(nc) as tc:
        with (
            tc.tile_pool(name="dram", bufs=1, space="DRAM") as dram,
            tc.tile_pool(name="apool", bufs=2) as apool,
            tc.tile_pool(name="bpool", bufs=16) as bpool,
            tc.tile_pool(name="cpool", bufs=4) as cpool,
            tc.tile_pool(name="psum", bufs=1, space="PSUM") as psum_pool,
        ):
            dram_io = {
                "apack": dram.tile(
                    [BLK, NSLOT, nplane, BLK], dt_in, kind="ExternalInput",
                    name="apack", uniquify=False,
                ),
                "bcat": dram.tile(
                    [N, nplane * N], dt_in, kind="ExternalInput",
                    name="bcat", uniquify=False,
                ),
                "cpart": dram.tile(
                    [MAXB * BLK, N], f32, kind="ExternalOutput",
                    name="cpart", uniquify=False,
                ),
            }
            pid = nc.partition_id()
            pools = (apool, bpool, cpool, psum_pool)
            for c in range(N_CORES):
                with tc.If(pid == c):
                    if repeat > 1:
                        with tc.For_i(
                            0, repeat, 1, hint_engines=tuple(mybir.ALL_ENGINES)
                        ):
                            _emit_core(nc, tc, pools, dram_io, c, mode, variant)
                    else:
                        _emit_core(nc, tc, pools, dram_io, c, mode, variant)
    nc.compile()
    return nc


_cached_nc = {}


def _get_nc(mode):
    if mode not in _cached_nc:
        _cached_nc[mode] = _build(mode)
    return _cached_nc[mode]


def _host_pack(A, B, mode):
    """Build per-core apack tensors and the shared bcat tensor."""
    if mode == "bf16x3":
        import ml_dtypes

        bf16 = ml_dtypes.bfloat16
        AT = np.ascontiguousarray(A.T)
        ath = AT.astype(bf16)
        atl = (AT - ath.astype(np.float32)).astype(bf16)
        bh_ = B.astype(bf16)
        bl_ = (B - bh_.astype(np.float32)).astype(bf16)
        planes_a = [ath, atl]
        bcat = np.concatenate([bh_, bl_], axis=1)
        npdt = bf16
    else:
        AT = np.ascontiguousarray(A.T)
        planes_a = [AT]
        bcat = np.ascontiguousarray(B)
        npdt = np.float32
    nplane = len(planes_a)

    apacks = []
    for c in range(N_CORES):
        ap = np.zeros((BLK, NSLOT, nplane, BLK), dtype=npdt)
        for I in BINS[c]:
            base = ABASE[c][I]
            for j, K in enumerate(range(I, NB)):
                for t, pl in enumerate(planes_a):
                    ap[:, base + j, t, :] = pl[
                        K * BLK : (K + 1) * BLK, I * BLK : (I + 1) * BLK
                    ]
        apacks.append(ap)
    return apacks, bcat


LAST_RESULT = None  # set by kernel(); test.py reads .exec_time_ns when tracing


def kernel(A, B):
    global LAST_RESULT
    from concourse.bass_utils import run_bass_kernel_spmd

    A = np.asarray(A, dtype=np.float32)
    B = np.asarray(B, dtype=np.float32)
    nc = _get_nc(MODE)
    apacks, bcat = _host_pack(A, B, MODE)
    in_maps = [{"apack": apacks[c], "bcat": bcat} for c in range(N_CORES)]
    res = run_bass_kernel_spmd(nc, in_maps, core_ids=list(range(N_CORES)))
    LAST_RESULT = res

    C = np.zeros((N, N), dtype=np.float32)
    for c in range(N_CORES):
        cp = res.results[c]["cpart"]
        for s, I in enumerate(sorted(BINS[c])):
            C[I * BLK : (I + 1) * BLK, I * BLK :] = cp[s * BLK : (s + 1) * BLK, I * BLK :]
    return C



# revision 6
# speedup vs baseline: 1.6439x; 1.6439x over previous
"""Triangular matmul C = triu(triu(A) @ triu(B)) on 8 TRN2 NeuronCores.

Structure: the (I, K, J) block-tetrahedron {I <= K <= J} (128x128 blocks,
N=4096 -> 32 blocks/side) is sharded by output row-block I across the 8
cores with a work-balanced assignment.  Each core runs its own statically
addressed program inside a `tc.If(partition_id == c)` block.

Per core: row-blocks are processed in groups of <=4 that share one sweep
over the B strips (B[K, K*128:] for K >= min(group)).  The J axis is cut
into 512-wide phases; each group member I gets one PSUM bank per phase
(double-buffered), accumulating A^T[K,I] @ B[K, phase-window] over K, then
evicting to the output.

Numerics: fp32 operands are split on the host into bf16 (hi, lo) pairs and
each block product uses 3 bf16 matmuls (Ah@Bh + Ah@Bl + Al@Bh), giving
~5e-6 relative error vs the fp32 reference at 3/4 the cost of the PE's
native 4-pass fp32 mode (measured on HW: rel_absmax 4.8e-6 at N=4096).

DMA discipline: one ~256KB DMA per (K-strip, phase) carrying hi and lo
planes together (Bcat = [Bh | Bl]), issued alternately from the SP and DVE
sequencers; A^T strips are host-packed per core (apack) so each strip is a
few large-line DMAs on the GpSimd sequencer; PSUM evictions copy on ACT and
store from its sequencer.  This keeps ~16 DMA engines busy without
sequencer issue serialization.

The kernel takes FULL (unsharded) inputs and returns the FULL output.
"""

import numpy as np

N = 4096
BLK = 128
NB = N // BLK  # 32
N_CORES = 8
PHASE = 512  # J-phase width (one PSUM bank of fp32)
MODE = "bf16x1"  # "bf16x1" | "bf16x3" | "fp32r" | "fp32"

# Work-balanced assignment of row-blocks I to cores (work(I) = T(32-I),
# T(m)=m(m+1)/2; bins balanced to 743..752 of 5984/8=748).
BINS = [
    [0, 14, 23],
    [1, 15, 21, 25, 29],
    [2, 13, 20, 28],
    [4, 12, 16],
    [3, 10, 22],
    [6, 9, 17, 30],
    [5, 11, 19, 24, 27, 31],
    [7, 8, 18, 26],
]
MAXB = max(len(b) for b in BINS)  # output row-slots per core
# A-pack slot layout: per core, the A^T strips (one 128x128 block per slot,
# hi+lo planes) for each owned I, K = I..31, concatenated.
ABASE = [
    {I: int(np.cumsum([0] + [NB - J for J in sorted(b)])[i]) for i, I in enumerate(sorted(b))}
    for b in BINS
]
NSLOT = 80  # >= max per-core total blocks (75)
ACHUNK = 4  # A-load DMA granularity in k-blocks


def _groups(bin_is):
    """Split a sorted bin into contiguous groups of <=4 minimizing the
    total B-strip traffic sum(T(32 - min(group)))."""
    Is = sorted(bin_is)
    t = lambda m: m * (m + 1) // 2
    best = None

    def rec(i, acc, parts):
        nonlocal best
        if i == len(Is):
            if best is None or acc < best[0]:
                best = (acc, [list(p) for p in parts])
            return
        for g in range(1, 5):
            if i + g <= len(Is):
                rec(i + g, acc + t(NB - Is[i]), parts + [Is[i : i + g]])

    rec(0, 0, [])
    return best[1]


def _emit_core(nc, tc, pools, dram_io, core, mode, variant="full"):
    """K-major schedule: one row-block I at a time, full output row in PSUM
    (8 banks), K-sweep with each A-tile's weights amortized over all J-chunks
    (weight switches are ~180ns on HW; this gives 2 per (I,K) instead of 2
    per (I,K,phase))."""
    apool, bpool, cpool, psum_pool = pools
    import concourse.mybir as mybir

    f32 = mybir.dt.float32
    nplane = 2 if mode == "bf16x3" else 1
    dt_in = {
        "bf16x1": mybir.dt.bfloat16,
        "bf16x3": mybir.dt.bfloat16,
        "fp32r": mybir.dt.float32r,
        "fp32": f32,
    }[mode]
    apack, bcat, cpart = dram_io["apack"], dram_io["bcat"], dram_io["cpart"]
    bcat3 = bcat.rearrange("k (t n) -> k t n", t=nplane)

    bin_is = BINS[core]
    slot = {I: s for s, I in enumerate(sorted(bin_is))}
    bdma_engines = [nc.sync, nc.scalar]
    bdma_i = 0

    static_b = None
    if "nobdma" in variant:
        static_b = []
        for ci in range(NB // 8):
            sb_t = bpool.tile(
                [BLK, nplane, 2 * PHASE], dt_in, name=f"sb_{ci}", tag=f"sb{ci}", bufs=1
            )
            nc.gpsimd.memset(sb_t[:], 0.5)
            static_b.append(sb_t)

    for I in sorted(bin_is):
        nblk = NB - I
        base = ABASE[core][I]
        a_t = apool.tile([BLK, nblk, nplane, BLK], dt_in, name=f"a_{I}", tag="a")
        for j0 in range(0, nblk, ACHUNK):
            j1 = min(j0 + ACHUNK, nblk)
            nc.gpsimd.dma_start(
                a_t[:, j0:j1, :, :], apack[:, base + j0 : base + j1, :, :]
            )
        c0 = I // 4  # first active PSUM bank / J-chunk
        ps = {
            c: psum_pool.tile([BLK, PHASE], f32, name=f"ps_{I}_{c}", tag=f"ps{c}")
            for c in range(c0, NB // 4)
        }

        for K in range(I, NB):
            kb = K - I
            # B strip double-chunks (1024 cols -> 2KB DMA lines; hi+lo planes
            # in one DMA).  Each plane feeds two 512-wide matmuls (PSUM bank
            # limit).
            b_ts = {}
            for d in range(K // 8, NB // 8):
                pstart = max(K * BLK, 2 * PHASE * d)
                width = 2 * PHASE * (d + 1) - pstart
                if "nobdma" in variant:
                    b_ts[d] = (static_b[d], pstart, width)
                    continue
                b_t = bpool.tile(
                    [BLK, nplane, 2 * PHASE], dt_in, name=f"b_{K}_{d}", tag="b"
                )
                for t in range(nplane):
                    eng = bdma_engines[bdma_i % len(bdma_engines)]
                    bdma_i += 1
                    eng.dma_start(
                        b_t[:, t, :width],
                        bcat3[K * BLK : (K + 1) * BLK, t, pstart : pstart + width],
                    )
                b_ts[d] = (b_t, pstart, width)

            first = K == I
            if mode == "bf16x3":
                passes = [(0, 0), (0, 1), (1, 0)]  # (A plane, B plane)
            else:
                passes = [(0, 0)]
            for pi, (ta, tb) in enumerate(passes):
                a_w = a_t[:, kb, ta, :]
                for c in range(K // 4, NB // 4):
                    d = c // 2
                    b_t, pstart, width = b_ts[d]
                    cstart = max(pstart, PHASE * c)  # global col of this MM
                    cwidth = PHASE * (c + 1) - cstart
                    o = ps[c][:, cstart - PHASE * c : PHASE]
                    boff = cstart - pstart  # offset into the b tile
                    is_first = first and pi == 0
                    is_last = pi == len(passes) - 1 and K == min(4 * c + 3, NB - 1)
                    if "nomm" not in variant:
                        nc.tensor.matmul(
                            o, a_w, b_t[:, tb, boff : boff + cwidth],
                            start=is_first, stop=is_last,
                        )

        # Evict the full output row; next I's banks free up as copies drain.
        for c in range(c0, NB // 4):
            if "nomm" in variant and "noevict" in variant:
                continue
            coff0 = max(I * BLK - PHASE * c, 0)
            w = PHASE - coff0
            ct = cpool.tile([BLK, PHASE], f32, name=f"c_{I}_{c}", tag="cst")
            nc.vector.tensor_copy(ct[:, :w], ps[c][:, coff0:PHASE])
            r0 = slot[I] * BLK
            nc.gpsimd.dma_start(
                cpart[r0 : r0 + BLK, PHASE * c + coff0 : PHASE * (c + 1)],
                ct[:, :w],
            )


def _build(mode, repeat=1, variant="full"):
    import concourse.mybir as mybir
    import concourse.tile as tile
    from concourse import bacc

    nc = bacc.Bacc(None, target_bir_lowering=False, debug=False)
    f32 = mybir.dt.float32
    nplane = 2 if mode == "bf16x3" else 1
    dt_in = {
        "bf16x1": mybir.dt.bfloat16,
        "bf16x3": mybir.dt.bfloat16,
        "fp32r": mybir.dt.float32r,
        "fp32": f32,
    }[mode]
    with tile.TileContext(nc) as tc:
        with (
            tc.tile_pool(name="dram", bufs=1, space="DRAM") as dram,
            tc.tile_pool(name="apool", bufs=2) as apool,
            tc.tile_pool(name="bpool", bufs=16) as bpool,
            tc.tile_pool(name="cpool", bufs=4) as cpool,
            tc.tile_pool(name="psum", bufs=1, space="PSUM") as psum_pool,
        ):
            dram_io = {
                "apack": dram.tile(
                    [BLK, NSLOT, nplane, BLK], dt_in, kind="ExternalInput",
                    name="apack", uniquify=False,
                ),
                "bcat": dram.tile(
                    [N, nplane * N], dt_in, kind="ExternalInput",
                    name="bcat", uniquify=False,
                ),
                "cpart": dram.tile(
                    [MAXB * BLK, N], f32, kind="ExternalOutput",
                    name="cpart", uniquify=False,
                ),
            }
            pid = nc.partition_id()
            pools = (apool, bpool, cpool, psum_pool)
            for c in range(N_CORES):
                with tc.If(pid == c):
                    if repeat > 1:
                        with tc.For_i(
                            0, repeat, 1, hint_engines=tuple(mybir.ALL_ENGINES)
                        ):
                            _emit_core(nc, tc, pools, dram_io, c, mode, variant)
                    else:
                        _emit_core(nc, tc, pools, dram_io, c, mode, variant)
    nc.compile()
    return nc


_cached_nc = {}


def _get_nc(mode):
    if mode not in _cached_nc:
        _cached_nc[mode] = _build(mode)
    return _cached_nc[mode]


def _host_pack(A, B, mode):
    """Build per-core apack tensors and the shared bcat tensor."""
    if mode == "bf16x3":
        import ml_dtypes

        bf16 = ml_dtypes.bfloat16
        AT = np.ascontiguousarray(A.T)
        ath = AT.astype(bf16)
        atl = (AT - ath.astype(np.float32)).astype(bf16)
        bh_ = B.astype(bf16)
        bl_ = (B - bh_.astype(np.float32)).astype(bf16)
        planes_a = [ath, atl]
        bcat = np.concatenate([bh_, bl_], axis=1)
        npdt = bf16
    elif mode == "bf16x1":
        import ml_dtypes

        bf16 = ml_dtypes.bfloat16
        AT = np.ascontiguousarray(A.T)
        planes_a = [AT.astype(bf16)]
        bcat = B.astype(bf16)
        npdt = bf16
    else:
        AT = np.ascontiguousarray(A.T)
        planes_a = [AT]
        bcat = np.ascontiguousarray(B)
        npdt = np.float32
    nplane = len(planes_a)

    apacks = []
    for c in range(N_CORES):
        ap = np.zeros((BLK, NSLOT, nplane, BLK), dtype=npdt)
        for I in BINS[c]:
            base = ABASE[c][I]
            for j, K in enumerate(range(I, NB)):
                for t, pl in enumerate(planes_a):
                    ap[:, base + j, t, :] = pl[
                        K * BLK : (K + 1) * BLK, I * BLK : (I + 1) * BLK
                    ]
        apacks.append(ap)
    return apacks, bcat


LAST_RESULT = None  # set by kernel(); test.py reads .exec_time_ns when tracing


def kernel(A, B):
    global LAST_RESULT
    from concourse.bass_utils import run_bass_kernel_spmd

    A = np.asarray(A, dtype=np.float32)
    B = np.asarray(B, dtype=np.float32)
    nc = _get_nc(MODE)
    apacks, bcat = _host_pack(A, B, MODE)
    in_maps = [{"apack": apacks[c], "bcat": bcat} for c in range(N_CORES)]
    res = run_bass_kernel_spmd(nc, in_maps, core_ids=list(range(N_CORES)))
    LAST_RESULT = res

    C = np.zeros((N, N), dtype=np.float32)
    for c in range(N_CORES):
        cp = res.results[c]["cpart"]
        for s, I in enumerate(sorted(BINS[c])):
            C[I * BLK : (I + 1) * BLK, I * BLK :] = cp[s * BLK : (s + 1) * BLK, I * BLK :]
    return C



# revision 8
# speedup vs baseline: 2.3311x; 1.4180x over previous
"""Triangular matmul C = triu(triu(A) @ triu(B)) on 8 TRN2 NeuronCores.

Schedule: the (I, K, J) block-tetrahedron {I <= K <= J} (128x128 blocks,
N=4096 -> 32 blocks/side) is decomposed into (quad, chunk) tasks, where a
quad q is 4 consecutive output row-blocks {4q..4q+3} and a chunk c is a
512-wide J-phase (one PSUM bank).  For a (q, c) pair the K sweep runs
4q..4c+3; each B strip B[K, chunk c] is DMA'd once and feeds 4 matmuls
(one per row in the quad), giving 4x B reuse over a row-at-a-time
schedule.  The 36 (q, c) pairs are assigned to the 8 cores by a
load-balance search over max(PE-stream, HBM) time; per-core W is 210-216
units (unit = one 128-deep K step of a 512-wide matmul, ~216 ns).

Numerics: single bf16 pass (operands rounded to bf16, fp32 PSUM
accumulation).  Measured 2.2e-3 relative absmax error vs the fp32
reference - inside the 2e-2 budget.  C is stored as bf16 (adds ~2^-9
relative) and upcast on the host.

Per core: ~216 matmuls + ~60 B DMAs (128 KB each, alternating SP/ACT
sequencers) + task-granular A DMAs (1 MB chunks on GpSimd) + 16-20 PSUM
evictions (DVE copy + store).  PSUM banks rotate 4+4 between chunk
phases so eviction drains behind the next phase's accumulation.

The kernel takes FULL (unsharded) inputs and returns the FULL output.
"""

import numpy as np

N = 4096
BLK = 128
NB = N // BLK  # 32
N_CORES = 8
PHASE = 512
MODE = "bf16x1"

# (quad, chunk) -> core assignment from the load-balance search:
# max per-core time ~46.7us with A<=6.5MB, B=7.5MB, W 210-216 units.
TASKS = [
    [(0, (2, 7)), (1, (1,)), (2, (3,)), (6, (6,))],
    [(0, (6,)), (3, (4, 6)), (6, (7,))],
    [(1, (2, 7)), (4, (5, 7))],
    [(0, (0, 3, 5)), (1, (3,)), (4, (4,))],
    [(1, (4, 6)), (2, (2, 4)), (7, (7,))],
    [(2, (5, 7)), (3, (5,)), (5, (6,))],
    [(0, (1, 4)), (2, (6,)), (4, (6,))],
    [(1, (5,)), (3, (3, 7)), (5, (5, 7))],
]
ACH = 8  # A-load DMA granularity in K-blocks (1MB per DMA)

NSLOT = max(
    sum(4 * (max(cs) - q + 1) for q, cs in tasks) for tasks in TASKS
)  # 52: A slots (one slot = 4 row-blocks of one K)
NCSLOT = max(sum(4 * len(cs) for _, cs in tasks) for tasks in TASKS)  # 20


def _emit_core(nc, tc, pools, dram_io, core):
    import concourse.mybir as mybir

    f32 = mybir.dt.float32
    bf16 = mybir.dt.bfloat16
    apool, bpool, cpool, psum_pool = pools
    apack, bcat, cpart = dram_io["apack"], dram_io["bcat"], dram_io["cpart"]

    bdma = [nc.sync, nc.scalar]
    cdma = [nc.gpsimd, nc.gpsimd]
    bdma_i = 0
    cdma_i = 0
    abase = 0
    cb = 0
    par = 0

    for ti, (q, cs) in enumerate(TASKS[core]):
        kext = 4 * (max(cs) - q + 1)
        a_ts = []
        for j0 in range(0, kext, ACH):
            jl = min(ACH, kext - j0)
            a_t = apool.tile(
                [BLK, jl, 4, BLK], bf16, name=f"a_{ti}_{j0}", tag=f"a{j0 // ACH}"
            )
            nc.gpsimd.dma_start(a_t[:], apack[:, abase + j0 : abase + j0 + jl, :, :])
            a_ts.append(a_t)
        abase += kext

        for c in sorted(cs):
            ps = [
                psum_pool.tile(
                    [BLK, PHASE], f32, name=f"ps_{ti}_{c}_{i}", tag=f"ps{i}_{par}"
                )
                for i in range(4)
            ]
            for K in range(4 * q, 4 * c + 4):
                kb = K - 4 * q
                off = max(0, K * BLK - PHASE * c)
                b_t = bpool.tile([BLK, PHASE], bf16, name=f"b_{ti}_{c}_{K}", tag="b")
                eng = bdma[bdma_i % 2]
                bdma_i += 1
                eng.dma_start(
                    b_t[:, off:PHASE],
                    bcat[K * BLK : (K + 1) * BLK, PHASE * c + off : PHASE * (c + 1)],
                )
                for i in range(4):
                    I = 4 * q + i
                    if K < I:
                        continue
                    nc.tensor.matmul(
                        ps[i][:, off:PHASE],
                        a_ts[kb // ACH][:, kb % ACH, i, :],
                        b_t[:, off:PHASE],
                        start=(K == I),
                        stop=(K == 4 * c + 3),
                    )
            for i in range(4):
                I = 4 * q + i
                coff = max(0, BLK * I - PHASE * c)
                w = PHASE - coff
                ct = cpool.tile([BLK, PHASE], bf16, name=f"c_{ti}_{c}_{i}", tag="cst")
                nc.vector.tensor_copy(ct[:, :w], ps[i][:, coff:PHASE])
                eng = cdma[cdma_i % 2]
                cdma_i += 1
                eng.dma_start(cpart[cb * BLK : (cb + 1) * BLK, 0:w], ct[:, :w])
                cb += 1
            par ^= 1


def _build():
    import concourse.mybir as mybir
    import concourse.tile as tile
    from concourse import bacc

    nc = bacc.Bacc(None, target_bir_lowering=False, debug=False)
    bf16 = mybir.dt.bfloat16
    with tile.TileContext(nc) as tc:
        with (
            tc.tile_pool(name="dram", bufs=1, space="DRAM") as dram,
            tc.tile_pool(name="apool", bufs=2) as apool,
            tc.tile_pool(name="bpool", bufs=16) as bpool,
            tc.tile_pool(name="cpool", bufs=4) as cpool,
            tc.tile_pool(name="psum", bufs=1, space="PSUM") as psum_pool,
        ):
            dram_io = {
                "apack": dram.tile(
                    [BLK, NSLOT, 4, BLK], bf16, kind="ExternalInput",
                    name="apack", uniquify=False,
                ),
                "bcat": dram.tile(
                    [N, N], bf16, kind="ExternalInput", name="bcat", uniquify=False,
                ),
                "cpart": dram.tile(
                    [NCSLOT * BLK, PHASE], bf16, kind="ExternalOutput",
                    name="cpart", uniquify=False,
                ),
            }
            pid = nc.partition_id()
            pools = (apool, bpool, cpool, psum_pool)
            for c in range(N_CORES):
                with tc.If(pid == c):
                    _emit_core(nc, tc, pools, dram_io, c)
    nc.compile()
    return nc


_cached_nc = {}


def _get_nc():
    if "v2" not in _cached_nc:
        _cached_nc["v2"] = _build()
    return _cached_nc["v2"]


def _host_pack(A, B):
    import ml_dtypes

    bf16 = ml_dtypes.bfloat16
    AT = np.ascontiguousarray(A.T).astype(bf16)
    bcat = np.ascontiguousarray(B.astype(bf16))

    apacks = []
    for core in range(N_CORES):
        ap = np.zeros((BLK, NSLOT, 4, BLK), dtype=bf16)
        abase = 0
        for q, cs in TASKS[core]:
            kext = 4 * (max(cs) - q + 1)
            for kb in range(kext):
                K = 4 * q + kb
                for i in range(4):
                    I = 4 * q + i
                    if K >= I:
                        ap[:, abase + kb, i, :] = AT[
                            K * BLK : (K + 1) * BLK, I * BLK : (I + 1) * BLK
                        ]
            abase += kext
        apacks.append(ap)
    return apacks, bcat


LAST_RESULT = None  # set by kernel(); test.py reads .exec_time_ns when tracing


def kernel(A, B):
    global LAST_RESULT
    from concourse.bass_utils import run_bass_kernel_spmd

    A = np.asarray(A, dtype=np.float32)
    B = np.asarray(B, dtype=np.float32)
    nc = _get_nc()
    apacks, bcat = _host_pack(A, B)
    in_maps = [{"apack": apacks[c], "bcat": bcat} for c in range(N_CORES)]
    res = run_bass_kernel_spmd(nc, in_maps, core_ids=list(range(N_CORES)))
    LAST_RESULT = res

    C = np.zeros((N, N), dtype=np.float32)
    for core in range(N_CORES):
        cp = res.results[core]["cpart"].astype(np.float32)
        cb = 0
        for q, cs in TASKS[core]:
            for c in sorted(cs):
                for i in range(4):
                    I = 4 * q + i
                    coff = max(0, BLK * I - PHASE * c)
                    w = PHASE - coff
                    C[
                        I * BLK : (I + 1) * BLK,
                        PHASE * c + coff : PHASE * (c + 1),
                    ] = cp[cb * BLK : cb * BLK + BLK, 0:w]
                    cb += 1
    return C


# revision 9
# speedup vs baseline: 3.2723x; 1.4038x over previous
"""Triangular matmul C = triu(triu(A) @ triu(B)) on 8 TRN2 NeuronCores.

Uniform-SPMD schedule: all 8 cores execute the IDENTICAL instruction
stream - no partition_id, no tc.If blocks (each If block costs ~6us of
inter-engine barrier/branch chain per core, ~45us total at 8 blocks).
Per-core work is defined purely by host-packed data.

Decomposition: the (I, K, J) block-tetrahedron {I <= K <= J} (128x128
blocks, N=4096 -> 32 blocks/side) is cut into 36 (quad, chunk) tasks -
quad q = output row-blocks {4q..4q+3}, chunk c = a 512-wide J-phase, K
sweep 4q..4c+3 so each B strip feeds 4 matmuls (4x B reuse).  Tasks are
then cut along K into pieces of extent {4,8,12,16} that exactly fill a
per-core template of 6 slots (4+16+8+16+12+4 = 60 K-blocks) - zero
padding, perfect 8-way balance by construction.  A slot accumulates its
piece in 4 PSUM banks (parity-alternating with the neighbour slots) and
evicts 4 partial [128,512] tiles; the host sums partials of split tasks
and applies the final triu mask (so no column trimming or skip masks are
needed on-device - zero-filled bf16 operand data handles the triangle).

Numerics: single bf16 pass (operands rounded to bf16, fp32 PSUM
accumulation), C partials stored as bf16: ~4e-3 relative absmax error vs
the fp32 reference (budget 2e-2).

Per core: 240 matmuls (512-wide, warm ~216 ns), 15 B-group DMAs + 15
A-group DMAs of 512KB each, 24 PSUM evictions.  HBM ~18 MB/core, PE
~52us - compute-bound at the bf16 PE roofline.

The kernel takes FULL (unsharded) inputs and returns the FULL output.
"""

import numpy as np

N = 4096
BLK = 128
NB = N // BLK  # 32
N_CORES = 8
PHASE = 512
MODE = "bf16x1-uniform"

TEMPLATE = (4, 16, 8, 16, 12, 4)  # slot K-extents; sum = 60
NUNIT = sum(TEMPLATE)  # 60 K-block units per core
NCSLOT = 4 * len(TEMPLATE)  # 24 C partial slots per core
GRP = 4  # DMA batch: 4 K-units per A/B transfer (512KB each)


def _mk_slots():
    """Cut the 36 (q,c) tasks into pieces filling 8 copies of TEMPLATE.

    Returns SLOTS[core] = list of (q, c, k0, ext) in template order."""
    from collections import Counter

    tasks = [(q, c) for q in range(8) for c in range(q, 8)]
    tasks.sort(key=lambda p: (-(p[1] - p[0]), p))  # L descending
    cap = Counter()
    for e in TEMPLATE:
        cap[e] += N_CORES
    by_ext = {e: [] for e in cap}
    for q, c in tasks:
        rem = 4 * (c - q) + 4
        k0 = 0
        while rem > 0:
            avail = sorted(e for e in cap if cap[e] > 0)
            le = [e for e in avail if e <= rem]
            e = le[-1] if le else avail[0]
            assert le, "template packing must be exact"
            cap[e] -= 1
            by_ext[e].append((q, c, k0))
            k0 += e
            rem -= e
    assert all(v == 0 for v in cap.values())
    # deal pieces to cores: template position order; same-extent positions
    # consume pieces in sequence
    used = {e: 0 for e in by_ext}
    slots = [[] for _ in range(N_CORES)]
    for core in range(N_CORES):
        for e in TEMPLATE:
            q, c, k0 = by_ext[e][used[e]]
            used[e] += 1
            slots[core].append((q, c, k0, e))
    return slots


SLOTS = _mk_slots()


def _emit(nc, tc, pools):
    import concourse.mybir as mybir

    f32 = mybir.dt.float32
    bf16 = mybir.dt.bfloat16
    apool, bpool, cpool, psum_pool, dram_io = pools
    apack, bpack, cpart = dram_io["apack"], dram_io["bpack"], dram_io["cpart"]

    bdma = [nc.sync, nc.scalar]
    a_ts = {}
    b_ts = {}

    # slot boundaries in global unit index
    starts = []
    u = 0
    for e in TEMPLATE:
        starts.append(u)
        u += e

    cb = 0
    for s, e in enumerate(TEMPLATE):
        u0 = starts[s]
        par = s % 2
        ps = [
            psum_pool.tile([BLK, PHASE], f32, name=f"ps_{s}_{i}", tag=f"ps{i}_{par}")
            for i in range(4)
        ]
        for u in range(u0, u0 + e):
            g = u // GRP
            if g not in b_ts:
                b_t = bpool.tile([BLK, GRP, PHASE], bf16, name=f"b_{g}", tag="b")
                bdma[g % 2].dma_start(b_t[:], bpack[:, g * GRP : (g + 1) * GRP, :])
                b_ts[g] = b_t
                a_t = apool.tile([BLK, GRP, 4, BLK], bf16, name=f"a_{g}", tag="a")
                nc.gpsimd.dma_start(a_t[:], apack[:, g * GRP : (g + 1) * GRP, :, :])
                a_ts[g] = a_t
            j = u % GRP
            for i in range(4):
                nc.tensor.matmul(
                    ps[i][:],
                    a_ts[g][:, j, i, :],
                    b_ts[g][:, j, :],
                    start=(u == u0),
                    stop=(u == u0 + e - 1),
                )
        for i in range(4):
            ct = cpool.tile([BLK, PHASE], bf16, name=f"c_{s}_{i}", tag="cst")
            nc.vector.tensor_copy(ct[:], ps[i][:])
            nc.gpsimd.dma_start(cpart[cb * BLK : (cb + 1) * BLK, :], ct[:])
            cb += 1


def _build():
    import concourse.mybir as mybir
    import concourse.tile as tile
    from concourse import bacc

    nc = bacc.Bacc(None, target_bir_lowering=False, debug=False)
    bf16 = mybir.dt.bfloat16
    with tile.TileContext(nc) as tc:
        with (
            tc.tile_pool(name="dram", bufs=1, space="DRAM") as dram,
            tc.tile_pool(name="apool", bufs=8) as apool,
            tc.tile_pool(name="bpool", bufs=8) as bpool,
            tc.tile_pool(name="cpool", bufs=4) as cpool,
            tc.tile_pool(name="psum", bufs=1, space="PSUM") as psum_pool,
        ):
            dram_io = {
                "apack": dram.tile(
                    [BLK, NUNIT, 4, BLK], bf16, kind="ExternalInput",
                    name="apack", uniquify=False,
                ),
                "bpack": dram.tile(
                    [BLK, NUNIT, PHASE], bf16, kind="ExternalInput",
                    name="bpack", uniquify=False,
                ),
                "cpart": dram.tile(
                    [NCSLOT * BLK, PHASE], bf16, kind="ExternalOutput",
                    name="cpart", uniquify=False,
                ),
            }
            _emit(nc, tc, (apool, bpool, cpool, psum_pool, dram_io))
    nc.compile()
    return nc


_cached_nc = {}


def _get_nc():
    if "v3" not in _cached_nc:
        _cached_nc["v3"] = _build()
    return _cached_nc["v3"]


def _host_pack(A, B):
    import ml_dtypes

    bf16 = ml_dtypes.bfloat16
    AT = np.ascontiguousarray(A.T).astype(bf16)
    Bb = np.ascontiguousarray(B.astype(bf16))

    apacks, bpacks = [], []
    for core in range(N_CORES):
        ap = np.zeros((BLK, NUNIT, 4, BLK), dtype=bf16)
        bp = np.zeros((BLK, NUNIT, PHASE), dtype=bf16)
        u = 0
        for q, c, k0, e in SLOTS[core]:
            for j in range(e):
                K = 4 * q + k0 + j
                bp[:, u, :] = Bb[K * BLK : (K + 1) * BLK, PHASE * c : PHASE * (c + 1)]
                for i in range(4):
                    I = 4 * q + i
                    if K >= I:
                        ap[:, u, i, :] = AT[
                            K * BLK : (K + 1) * BLK, I * BLK : (I + 1) * BLK
                        ]
                u += 1
        apacks.append(ap)
        bpacks.append(bp)
    return apacks, bpacks


LAST_RESULT = None  # set by kernel(); test.py reads .exec_time_ns when tracing


def kernel(A, B):
    global LAST_RESULT
    from concourse.bass_utils import run_bass_kernel_spmd

    A = np.asarray(A, dtype=np.float32)
    B = np.asarray(B, dtype=np.float32)
    nc = _get_nc()
    apacks, bpacks = _host_pack(A, B)
    in_maps = [{"apack": apacks[c], "bpack": bpacks[c]} for c in range(N_CORES)]
    res = run_bass_kernel_spmd(nc, in_maps, core_ids=list(range(N_CORES)))
    LAST_RESULT = res

    C = np.zeros((N, N), dtype=np.float32)
    for core in range(N_CORES):
        cp = res.results[core]["cpart"].astype(np.float32)
        for sidx, (q, c, k0, e) in enumerate(SLOTS[core]):
            for i in range(4):
                I = 4 * q + i
                C[I * BLK : (I + 1) * BLK, PHASE * c : PHASE * (c + 1)] += cp[
                    (sidx * 4 + i) * BLK : (sidx * 4 + i + 1) * BLK, :
                ]
    return np.triu(C)
